# revision 7
# baseline (speedup 1.0000x reference)
"""Trainium2 Bass kernel: pre-norm transformer encoder block (B=2,N=2048,D=1024,
Hid=4096,H=16 heads, raw-reshape attention merge, shared LN params).

Sharding (8 cores, no collectives):
  core c: b = c//4, heads h = 4*(c%4)..4*(c%4)+3 of batch b.
  The raw o.reshape(B,N,D) merge maps head h exactly onto tokens
  [128h, 128h+128) of the residual stream, so each core's attention output
  lands on its own contiguous 512-token slice -> MLP is token-parallel with
  zero communication.

v2: all matmuls bf16 (FWL-eligible weights), LayerNorm normalize-once with
PE outer-product broadcasts (no DRAM roundtrips in phase A), fast approx
reciprocal for softmax denominators, and the first MLP token-half issue-
interleaved into the second attention head-pair so the tensor engine stays
busy while the scalar engine runs the softmax Exp stream.
"""

from contextlib import ExitStack

import numpy as np
import ml_dtypes
import bass_rust
import concourse.bass as bass
import concourse.mybir as mybir
from concourse.tile import TileContext, ScopedClock
from concourse.bass import ts

F32 = mybir.dt.float32
F32R = mybir.dt.float32r
BF16 = mybir.dt.bfloat16
AF = mybir.ActivationFunctionType
OP = mybir.AluOpType

B, N, D, HID, H = 2, 2048, 1024, 4096, 16
DH = D // H            # 64
NCORES = 8
CPB = 4                # cores per batch
NH = 4                 # heads per core
TOK = N                # tokens per batch (attention span)
MY = 512               # tokens owned per core (MLP/residual)
P = 128
SL = 512               # free-dim slice for matmuls
NSL = TOK // SL        # 4
KD = D // P            # 8
NKT = TOK // P         # 16
HT = HID // P          # 32
HHALF = MY // 2        # 256 tokens per MLP half
EPS = 1e-5
EXP_SHIFT = -20.0      # constant logit shift; cancels in softmax, guards overflow

_PATCHED = False


def _patch_drain():
    """This walrus build rejects >2 sem waits on one instruction; split the
    Tile kernel-tail drain's waits across single-wait NOPs."""
    global _PATCHED
    if _PATCHED:
        return
    _PATCHED = True

    def _drain_and_barrier(self, tick_clock, wait_clock):
        gc = tick_clock.global_clock
        ticks = eval(repr(gc).replace("VectorClock", ""))
        n = len(ticks)
        for i, t in enumerate(ticks):
            if t > 0:
                single = [0] * n
                single[i] = t
                vc = bass_rust.VectorClock(single)
                nop = self.nc.sync.nop(nofuse=True, hint=f"drain_split_{i}")
                wait_clock.add_sem_waits(nop.ins, ScopedClock({None: vc}))
        self.nc.sync.drain()
        self.nc.all_engine_barrier()
        assert self.sems is not None
        popped = self.nc._tile_sem_poison_stack.pop()
        assert popped is self._sem_poison
        self.nc.clear_and_free_semaphores(list(self.sems.allocated().values()))
        self.nc.all_engine_barrier()

    TileContext._drain_and_barrier = _drain_and_barrier


def _split_excess_waits(nc):
    """This walrus build supports only one sync wait per instruction (two for
    EventSemaphore). Tile emits more; move the excess onto injected NoOps that
    run just before the instruction on the same engine."""
    nid = [0]
    for fn in nc.m.functions:
        for bb in fn.blocks:
            out = []
            changed = False
            for inst in bb.instructions:
                si = inst.sync_info
                waits = list(si.on_wait) if si is not None and si.on_wait else []
                cap = 2 if inst.opcode == "EventSemaphore" else 1
                if len(waits) > cap:
                    changed = True
                    for w in waits[:-cap]:
                        nid[0] += 1
                        nop = bass_rust.InstNoOp(
                            name=f"I-wsplit{nid[0]}", ins=[], outs=[])
                        nop.engine = inst.engine
                        nop.sync_info = bass_rust.SyncInfo(
                            on_wait=[w], on_update=[])
                        out.append(nop)
                    ups = list(si.on_update) if si.on_update else []
                    inst.sync_info = bass_rust.SyncInfo(
                        on_wait=waits[-cap:], on_update=ups)
                out.append(inst)
            if changed:
                bb.instructions = out


def build_program(split_waits=True):
    _patch_drain()
    rr = lambda ap: ap.bitcast(F32R)
    nc = bass.Bass()

    xT = nc.dram_tensor("xT", [D, TOK], BF16, kind="ExternalInput")
    xTmy = nc.dram_tensor("xTmy", [D, MY], F32, kind="ExternalInput")
    wqkv = nc.dram_tensor("wqkv", [D, 768], BF16, kind="ExternalInput")
    bqk = nc.dram_tensor("bqk", [4 * P], F32, kind="ExternalInput")
    bv = nc.dram_tensor("bv", [NH * DH], F32, kind="ExternalInput")
    w1 = nc.dram_tensor("w1", [D, HID], BF16, kind="ExternalInput")
    b1 = nc.dram_tensor("b1", [HID], F32, kind="ExternalInput")
    w2 = nc.dram_tensor("w2", [HID, D], BF16, kind="ExternalInput")
    b2 = nc.dram_tensor("b2", [D], F32, kind="ExternalInput")
    ones_in = nc.dram_tensor("ones_in", [P], F32R, kind="ExternalInput")
    outT = nc.dram_tensor("outT", [D, MY], F32, kind="ExternalOutput")

    # scratch DRAM for softmax-denominator broadcast roundtrips
    scr_rcp = nc.dram_tensor("scr_rcp", [16, SL], F32)

    with TileContext(nc) as tc, ExitStack() as top:
        singles = top.enter_context(tc.tile_pool(name="singles", bufs=1))
        xTmy_pool = top.enter_context(tc.tile_pool(name="xTmyp", bufs=1))
        x2T_pool = top.enter_context(tc.tile_pool(name="x2T", bufs=1))

        onesP_row = singles.tile([1, P], F32R)
        nc.sync.dma_start(out=onesP_row, in_=ones_in[None, :])
        ones_bf = singles.tile([P, 1], BF16)
        nc.vector.memset(ones_bf, 1.0)
        ones_f32 = singles.tile([P, 1], F32R)
        nc.sync.dma_start(out=ones_f32, in_=ones_in[:, None])
        eps1 = singles.tile([1, 1], F32)
        nc.vector.memset(eps1, EPS)
        shiftP = singles.tile([P, 1], F32)
        nc.vector.memset(shiftP, EXP_SHIFT)
        bqk_sb = singles.tile([P, 4], F32)
        nc.sync.dma_start(out=bqk_sb, in_=bqk.rearrange("(c p) -> p c", p=P))
        bvB = singles.tile([P, NH * DH], F32)
        nc.sync.dma_start(out=bvB, in_=bv[None, :].to_broadcast([P, NH * DH]))
        b1_sb = singles.tile([P, HT], F32)
        nc.sync.dma_start(out=b1_sb, in_=b1.rearrange("(c p) -> p c", p=P))
        b2_sb = singles.tile([P, KD], F32)
        nc.sync.dma_start(out=b2_sb, in_=b2.rearrange("(c p) -> p c", p=P))

        xTmy_t = xTmy_pool.tile([P, KD * MY], F32)   # col = 512k + m
        for k in range(KD):
            nc.sync.dma_start(out=xTmy_t[:, ts(k, MY)], in_=xTmy[ts(k, P), :])
        x2T = x2T_pool.tile([P, KD * MY], F32R)      # x + attn_out, same layout

        # pools that live from phase A into phase B
        esAB = ExitStack()
        qkvT_pool = esAB.enter_context(tc.tile_pool(name="qkvT", bufs=1))
        vsb_pool = esAB.enter_context(tc.tile_pool(name="vsb", bufs=1))

        # qkvT col-tiles: 0=[q_h0;q_h1] 1=[q_h2;q_h3] 2=[k_h0;k_h1] 3=[k_h2;k_h3]
        qkvT = [qkvT_pool.tile([P, TOK], BF16, name=f"qkvT{ct}", tag=f"qkvT{ct}")
                for ct in range(4)]
        # per nk token-tile: [tok, (h dh+1)] with a ones column per head
        vsb = [vsb_pool.tile([P, NH * (DH + 1)], BF16, name=f"v{nk}",
                             tag=f"v{nk}") for nk in range(NKT)]

        # ================= Phase A: LN1 + qkv (per-sl groups) ================
        with ExitStack() as esA:
            wqkv_pool = esA.enter_context(tc.tile_pool(name="wqkvp", bufs=1))
            xts_pool = esA.enter_context(tc.tile_pool(name="xts", bufs=2))
            xsq_pool = esA.enter_context(tc.tile_pool(name="xsq", bufs=2))
            xn_pool = esA.enter_context(tc.tile_pool(name="xn", bufs=2))
            rows_pool = esA.enter_context(tc.tile_pool(name="rows", bufs=2))
            bcA_pool = esA.enter_context(tc.tile_pool(name="bcA", bufs=2))
            psA = esA.enter_context(tc.tile_pool(name="psA", bufs=2, space="PSUM"))
            psB = esA.enter_context(tc.tile_pool(name="psB", bufs=1, space="PSUM"))
            psQ = esA.enter_context(tc.tile_pool(name="psQ", bufs=1, space="PSUM"))
            psV = esA.enter_context(tc.tile_pool(name="psV", bufs=1, space="PSUM"))

            wqkv_sb = []
            for k in range(KD):
                t = wqkv_pool.tile([P, 768], BF16, tag=f"wqkv{k}")
                nc.sync.dma_start(out=t, in_=wqkv[ts(k, P), :])
                wqkv_sb.append(t)

            def issue_load(sl):
                xts = []
                for k in range(KD):
                    t = xts_pool.tile([P, SL], BF16, name="xts", tag=f"xts{k}")
                    nc.sync.dma_start(out=t, in_=xT[ts(k, P), ts(sl, SL)])
                    xts.append(t)
                return xts

            def issue_stats(xts):
                s1p = psA.tile([1, SL], F32, tag="s1")
                s2p = psA.tile([1, SL], F32, tag="s2")
                for k in range(KD):
                    xsq = xsq_pool.tile([P, SL], BF16, name="xsq", tag="xsq")
                    nc.vector.tensor_mul(xsq, xts[k], xts[k])
                    nc.tensor.matmul(s1p, lhsT=ones_bf, rhs=xts[k],
                                     start=(k == 0), stop=(k == KD - 1))
                    nc.tensor.matmul(s2p, lhsT=ones_bf, rhs=xsq,
                                     start=(k == 0), stop=(k == KD - 1))
                return s1p, s2p

            loaded = issue_load(0)
            stats = issue_stats(loaded)
            for sl in range(NSL):
                xts, (s1p, s2p) = loaded, stats
                if sl + 1 < NSL:
                    loaded = issue_load(sl + 1)
                    stats = issue_stats(loaded)

                # --- row chain: rstd / -mu*rstd ---
                rows2 = rows_pool.tile([1, 2 * SL], F32R, tag="rows2")
                mus = rows_pool.tile([1, SL], F32, tag="mus")
                nc.scalar.activation(out=mus, in_=s1p, func=AF.Copy,
                                     bias=0.0, scale=1.0 / D)
                m2 = rows_pool.tile([1, SL], F32, tag="m2")
                nc.scalar.activation(out=m2, in_=s2p, func=AF.Copy,
                                     bias=0.0, scale=1.0 / D)
                mu2 = rows_pool.tile([1, SL], F32, tag="mu2")
                nc.vector.tensor_mul(mu2, mus, mus)
                var = rows_pool.tile([1, SL], F32, tag="var")
                nc.vector.tensor_sub(var, m2, mu2)
                sd = rows_pool.tile([1, SL], F32, tag="sd")
                nc.scalar.activation(out=sd, in_=var, func=AF.Sqrt,
                                     bias=eps1, scale=1.0)
                rstd_tmp = rows_pool.tile([1, SL], F32, tag="rstd_tmp")
                nc.vector.reciprocal(rstd_tmp, sd)
                nc.vector.tensor_scalar_mul(rows2[:, 0:SL], rstd_tmp, 1.0)
                mr = rows_pool.tile([1, SL], F32, tag="mr")
                nc.vector.tensor_mul(mr, mus, rstd_tmp)
                nc.vector.tensor_scalar_mul(rows2[:, SL:2 * SL], mr, -1.0)

                # --- broadcast via PE outer-product (cols 0:512 rstd,
                #     512:1024 -mu*rstd), then one ACT copy to bf16 SBUF ---
                ob = psB.tile([P, 2 * SL], F32, tag="ob")
                nc.tensor.matmul(ob[:, 0:SL], lhsT=onesP_row,
                                 rhs=rows2[:, 0:SL], start=True, stop=True,
                                 skip_group_check=True)
                nc.tensor.matmul(ob[:, SL:2 * SL], lhsT=onesP_row,
                                 rhs=rows2[:, SL:2 * SL], start=True,
                                 stop=True, skip_group_check=True)
                rb = bcA_pool.tile([P, 2 * SL], BF16, tag="rb")
                nc.scalar.activation(out=rb, in_=ob, func=AF.Copy,
                                     bias=0.0, scale=1.0)

                # --- normalize: xn = x*rstd + (-mu*rstd) in bf16 ---
                xn = []
                for k in range(KD):
                    tmp = xsq_pool.tile([P, SL], BF16, name="nt", tag="nt")
                    nc.vector.tensor_mul(tmp, xts[k], rb[:, 0:SL])
                    t = xn_pool.tile([P, SL], BF16, name="xn", tag=f"xn{k}")
                    nc.vector.tensor_add(t, tmp, rb[:, SL:2 * SL])
                    xn.append(t)

                # --- k, q projections (k first so attention can start asap) --
                for ct in (2, 3, 0, 1):
                    pq = psQ.tile([P, SL], F32, tag="pq")
                    for k in range(KD):
                        nc.tensor.matmul(
                            pq, lhsT=wqkv_sb[k][:, ts(ct, P)], rhs=xn[k],
                            start=(k == 0), stop=(k == KD - 1))
                    nc.scalar.activation(
                        out=qkvT[ct][:, ts(sl, SL)], in_=pq, func=AF.Identity,
                        bias=bqk_sb[:, ct:ct + 1], scale=1.0)

                # --- v projection ---
                for nkl in range(SL // P):
                    nk = (SL // P) * sl + nkl
                    pv = psV.tile([P, NH * DH], F32, tag="pv")
                    for k in range(KD):
                        nc.tensor.matmul(
                            pv, lhsT=xn[k][:, ts(nkl, P)],
                            rhs=wqkv_sb[k][:, 512:768],
                            start=(k == 0), stop=(k == KD - 1))
                    vt = vsb[nk]
                    vtv = vt.rearrange("p (h j) -> p h j", h=NH)
                    nc.vector.tensor_add(
                        vtv[:, :, 0:DH],
                        pv.rearrange("p (h j) -> p h j", h=NH),
                        bvB.rearrange("p (h j) -> p h j", h=NH))
                    nc.vector.memset(vtv[:, :, DH:DH + 1], 1.0)

        # ============ Phase B: attention (+ MLP half 0 interleaved) ==========
        # MLP work is issued as closures pumped between attention groups so
        # the in-order engine queues interleave MLP matmuls under the Exp
        # stream of head-pair 1.
        esB = ExitStack()
        psS = esB.enter_context(tc.tile_pool(name="psS", bufs=2, space="PSUM"))
        psO = esB.enter_context(tc.tile_pool(name="psO", bufs=1, space="PSUM"))
        psF = esB.enter_context(tc.tile_pool(name="psF", bufs=2, space="PSUM"))
        pt_pool = esB.enter_context(tc.tile_pool(name="pt", bufs=4))
        oT_pool = esB.enter_context(tc.tile_pool(name="oT", bufs=1))
        rcp_pool = esB.enter_context(tc.tile_pool(name="rcp", bufs=2))
        # MLP pools
        x2b_pool = esB.enter_context(tc.tile_pool(name="x2b", bufs=1))
        rows2_pool = esB.enter_context(tc.tile_pool(name="rows2", bufs=1))
        bc2_pool = esB.enter_context(tc.tile_pool(name="bc2", bufs=1))
        sq2_pool = esB.enter_context(tc.tile_pool(name="sq2", bufs=2))
        w1_pool = esB.enter_context(tc.tile_pool(name="w1sb", bufs=2))
        w2_pool = esB.enter_context(tc.tile_pool(name="w2sb", bufs=2))
        hT_pool = esB.enter_context(tc.tile_pool(name="hT", bufs=1))
        fctmp_pool = esB.enter_context(tc.tile_pool(name="fctmp", bufs=2))

        x2b = x2b_pool.tile([P, KD * MY], BF16)      # col = 512k + m
        w2r = w2.rearrange("(c p) d -> p c d", p=P)  # [128, 32, 1024]

        def mlp_half_units(half):
            """Yield issue-closures for the MLP on my-token half `half`
            (token cols [256*half, 256*half+256) of each 512-col k block)."""
            c0 = HHALF * half
            xsl = lambda t, k: t[:, k * MY + c0:k * MY + c0 + HHALF]
            w1sb = []
            w2q = []
            hts = {}

            def ln2_and_norm():
                tps = psS.tile([P, 2 * SL], F32, name="ln2s", tag="ps2")
                s1 = tps[0:1, 0:HHALF]
                s2 = tps[0:1, HHALF:2 * HHALF]
                for k in range(KD):
                    xx = xsl(x2T, k)
                    xxf = xx.bitcast(F32)
                    sq = sq2_pool.tile([P, HHALF], F32R, name="sq2", tag="sq2")
                    nc.vector.tensor_mul(sq, xxf, xxf)
                    nc.tensor.matmul(s1, lhsT=ones_f32, rhs=xx,
                                     start=(k == 0), stop=(k == KD - 1),
                                     skip_group_check=True)
                    nc.tensor.matmul(s2, lhsT=ones_f32, rhs=sq,
                                     start=(k == 0), stop=(k == KD - 1),
                                     skip_group_check=True)
                # row chain on [1, 256]
                rows2b = rows2_pool.tile([1, 2 * HHALF], F32R, tag="r2b")
                mus = rows2_pool.tile([1, HHALF], F32, tag="mus2")
                nc.scalar.activation(out=mus, in_=s1, func=AF.Copy,
                                     bias=0.0, scale=1.0 / D)
                m2 = rows2_pool.tile([1, HHALF], F32, tag="m22")
                nc.scalar.activation(out=m2, in_=s2, func=AF.Copy,
                                     bias=0.0, scale=1.0 / D)
                mu2 = rows2_pool.tile([1, HHALF], F32, tag="mu22")
                nc.vector.tensor_mul(mu2, mus, mus)
                var = rows2_pool.tile([1, HHALF], F32, tag="var2")
                nc.vector.tensor_sub(var, m2, mu2)
                sd = rows2_pool.tile([1, HHALF], F32, tag="sd2")
                nc.scalar.activation(out=sd, in_=var, func=AF.Sqrt,
                                     bias=eps1, scale=1.0)
                rstd2_tmp = rows2_pool.tile([1, HHALF], F32, tag="rs2t")
                nc.vector.reciprocal(rstd2_tmp, sd)
                nc.vector.tensor_scalar_mul(rows2b[:, 0:HHALF], rstd2_tmp, 1.0)
                mr = rows2_pool.tile([1, HHALF], F32, tag="mr2")
                nc.vector.tensor_mul(mr, mus, rstd2_tmp)
                nc.vector.tensor_scalar_mul(rows2b[:, HHALF:2 * HHALF],
                                            mr, -1.0)
                # broadcast outer-product into the ps2 scratch cols 512:1024
                obr = tps[:, SL:2 * SL]
                nc.tensor.matmul(obr, lhsT=onesP_row, rhs=rows2b,
                                 start=True, stop=True, skip_group_check=True)
                rb2 = bc2_pool.tile([P, SL], BF16, tag="rb2")
                nc.scalar.activation(out=rb2, in_=obr, func=AF.Copy,
                                     bias=0.0, scale=1.0)
                # normalize into x2b (bf16)
                for k in range(KD):
                    tmp = sq2_pool.tile([P, HHALF], BF16, name="n2", tag="n2")
                    nc.vector.tensor_mul(tmp, xsl(x2T, k).bitcast(F32),
                                         rb2[:, 0:HHALF])
                    nc.vector.tensor_add(xsl(x2b, k), tmp,
                                         rb2[:, HHALF:2 * HHALF])
            yield ln2_and_norm

            GK, GW = 8, HID // 8   # w1 col groups of 512
            for gk in range(GK):
                def load_w1(gk=gk):
                    w1sb.clear()
                    for k in range(KD):
                        t = w1_pool.tile([P, GW], BF16, name="w1t",
                                         tag=f"w1_{k}")
                        nc.sync.dma_start(out=t, in_=w1[ts(k, P), ts(gk, GW)])
                        w1sb.append(t)
                yield load_w1
                for khl in range(GW // P):
                    def fc1_unit(khl=khl, gk=gk):
                        kh = (GW // P) * gk + khl
                        pf = psF.tile([P, HHALF], F32, tag="pf")
                        for k in range(KD):
                            nc.tensor.matmul(
                                pf, lhsT=w1sb[k][:, ts(khl, P)],
                                rhs=xsl(x2b, k),
                                start=(k == 0), stop=(k == KD - 1))
                        ht = hT_pool.tile([P, MY], BF16, name="ht",
                                          tag=f"hT{kh}")
                        hts[kh] = ht
                        nc.scalar.activation(out=ht[:, c0:c0 + HHALF], in_=pf,
                                             func=AF.Gelu,
                                             bias=b1_sb[:, kh:kh + 1],
                                             scale=1.0)
                    yield fc1_unit

            for kd in range(KD):
                def load_w2(kd=kd):
                    t = w2_pool.tile([P, HT * P], BF16, name="w2t", tag="w2sb")
                    nc.sync.dma_start(
                        out=t.rearrange("p (c d) -> p c d", c=HT),
                        in_=w2r[:, :, ts(kd, P)])
                    w2q.append(t)
                def fc2_unit(kd=kd):
                    w2h = w2q.pop(0)
                    pf = psF.tile([P, HHALF], F32, tag="pf")
                    for kh in range(HT):
                        nc.tensor.matmul(
                            pf, lhsT=w2h[:, ts(kh, P)],
                            rhs=hts[kh][:, c0:c0 + HHALF],
                            start=(kh == 0), stop=(kh == HT - 1))
                    tb = fctmp_pool.tile([P, HHALF], F32, tag="fco")
                    nc.vector.tensor_scalar(
                        out=tb, in0=pf, scalar1=b2_sb[:, kd:kd + 1],
                        scalar2=None, op0=OP.add)
                    ot = fctmp_pool.tile([P, HHALF], F32, tag="fcout")
                    nc.vector.tensor_add(ot, tb, xsl(x2T, kd).bitcast(F32))
                    nc.sync.dma_start(out=outT[ts(kd, P), c0:c0 + HHALF],
                                      in_=ot)
                yield load_w2
                yield fc2_unit

        pending = []   # MLP closures awaiting issue

        def pump(n=1):
            for _ in range(n):
                if pending:
                    pending.pop(0)()

        for pair in range(2):
            qq = qkvT[pair]
            kk = qkvT[2 + pair]
            oTs2 = [oT_pool.tile([P, TOK], BF16, name=f"oTs{h}", tag=f"oT{h}")
                    for h in range(2)]
            for sl in range(NSL):
                po2 = [psO.tile([DH + 1, SL], F32, name=f"po{h}", tag=f"po{h}")
                       for h in range(2)]
                for nk in range(NKT):
                    ps2 = psS.tile([P, 2 * SL], F32, name="ps2", tag="ps2")
                    nc.tensor.matmul(
                        ps2[:, 0:SL], lhsT=kk[0:64, ts(nk, P)],
                        rhs=qq[0:64, ts(sl, SL)],
                        start=True, stop=True, tile_position=(0, 0))
                    nc.tensor.matmul(
                        ps2[:, SL:2 * SL], lhsT=kk[64:128, ts(nk, P)],
                        rhs=qq[64:128, ts(sl, SL)],
                        start=True, stop=True, tile_position=(64, 0))
                    pt2 = pt_pool.tile([P, 2 * SL], BF16, name="pt2", tag="pt2")
                    nc.scalar.activation(out=pt2, in_=ps2, func=AF.Exp,
                                         bias=shiftP, scale=1.0)
                    for h in range(2):
                        hh = 2 * pair + h
                        nc.tensor.matmul(
                            po2[h],
                            lhsT=vsb[nk][:, hh * (DH + 1):(hh + 1) * (DH + 1)],
                            rhs=pt2[:, ts(h, SL)],
                            start=(nk == 0), stop=(nk == NKT - 1))
                    pump(1)
                # denominators -> 1/den broadcast -> normalized oTs rows 0:64
                pous = []
                for h in range(2):
                    pou = rcp_pool.tile([DH + 1, SL], F32, tag=f"pou{h}")
                    nc.vector.tensor_copy(pou, po2[h])
                    pous.append(pou)
                for h in range(2):
                    idx = 8 * pair + 2 * sl + h
                    rcp_row = rcp_pool.tile([1, SL], F32, tag="rcp_row")
                    nc.vector.reciprocal(rcp_row, pous[h][DH:DH + 1, :])
                    nc.sync.dma_start(out=scr_rcp[idx], in_=rcp_row)
                    rcpB = rcp_pool.tile([DH, SL], F32, tag="rcpB")
                    nc.sync.dma_start(
                        out=rcpB,
                        in_=scr_rcp[idx][None, :].to_broadcast([DH, SL]))
                    oTs = oTs2[h]
                    nc.vector.tensor_mul(oTs[0:64, ts(sl, SL)],
                                         pous[h][0:DH, :], rcpB)
                    nc.sync.dma_start(out=oTs[64:128, ts(sl, SL)],
                                      in_=oTs[0:64, ts(sl, SL)])
                pump(1)
            # scatter: x2 = x + attn_out for this pair's two head-chunks
            # attn_out^T[64j+d, m] = oT[d, 16m+j]; oT col = 16m + 2*jj + two
            for h in range(2):
                hh = 2 * pair + h
                c0 = P * hh
                ov = oTs2[h].rearrange("p (m j two) -> p two j m", j=8, two=2)
                xv = xTmy_t.rearrange("p (k c) -> p k c", k=KD)
                x2v = x2T.rearrange("p (k c) -> p k c", k=KD)
                nc.vector.tensor_add(
                    x2v[0:64, :, c0:c0 + P], xv[0:64, :, c0:c0 + P],
                    ov[0:64, 0, :, :])
                nc.vector.tensor_add(
                    x2v[64:128, :, c0:c0 + P], xv[64:128, :, c0:c0 + P],
                    ov[64:128, 1, :, :])
            if pair == 0:
                pending.extend(mlp_half_units(0))
        while pending:
            pump(1)
        for unit in mlp_half_units(1):
            unit()
        esB.close()
        esAB.close()

    if split_waits:
        _split_excess_waits(nc)
    return nc


def host_prep(x, w_qkv, b_qkv, ln_g, ln_b, w1, b1, w2, b2):
    """Fold LN affine params into weights; build per-core input maps."""
    x = np.asarray(x, np.float32)
    w_qkv = np.asarray(w_qkv, np.float32)
    b_qkv = np.asarray(b_qkv, np.float32)
    ln_g = np.asarray(ln_g, np.float32)
    ln_b = np.asarray(ln_b, np.float32)
    w1 = np.asarray(w1, np.float32)
    b1 = np.asarray(b1, np.float32)
    w2 = np.asarray(w2, np.float32)
    b2 = np.asarray(b2, np.float32)

    wqkv_eff = ln_g[:, None] * w_qkv
    bqkv_eff = b_qkv + ln_b @ w_qkv
    w1_eff = np.ascontiguousarray(ln_g[:, None] * w1).astype(ml_dtypes.bfloat16)
    b1_eff = b1 + ln_b @ w1
    w2_bf = w2.astype(ml_dtypes.bfloat16)

    in_maps = []
    for c in range(NCORES):
        b = c // CPB
        heads = [NH * (c % CPB) + i for i in range(NH)]
        qcols = np.concatenate([np.arange(h * DH, (h + 1) * DH) for h in heads])
        kcols = qcols + D
        vcols = qcols + 2 * D
        allcols = np.concatenate([qcols, kcols, vcols])
        xb = x[b]
        my0 = MY * (c % CPB)
        in_maps.append({
            "ones_in": np.ones(P, np.float32),
            "xT": np.ascontiguousarray(xb.T).astype(ml_dtypes.bfloat16),
            "xTmy": np.ascontiguousarray(xb[my0:my0 + MY].T),
            "wqkv": np.ascontiguousarray(wqkv_eff[:, allcols]).astype(
                ml_dtypes.bfloat16),
            "bqk": np.ascontiguousarray(
                bqkv_eff[np.concatenate([qcols, kcols])]),
            "bv": np.ascontiguousarray(bqkv_eff[vcols]),
            "w1": w1_eff,
            "b1": b1_eff,
            "w2": w2_bf,
            "b2": b2,
        })
    return in_maps


_NC_CACHE = None


def kernel(x, w_qkv, b_qkv, ln_g, ln_b, w1, b1, w2, b2):
    global _NC_CACHE
    from concourse.bass_utils import run_bass_kernel_spmd

    if _NC_CACHE is None:
        _NC_CACHE = build_program()
    nc = _NC_CACHE
    in_maps = host_prep(x, w_qkv, b_qkv, ln_g, ln_b, w1, b1, w2, b2)
    res = run_bass_kernel_spmd(nc, in_maps, list(range(NCORES))).results

    out = np.empty((B, N, D), np.float32)
    for c in range(NCORES):
        b = c // CPB
        my0 = MY * (c % CPB)
        out[b, my0:my0 + MY, :] = res[c]["outT"].T
    return out
